# revision 1
# baseline (speedup 1.0000x reference)
"""Bass/Trainium2 kernel for nn_DecoderLSTM: batched decoder LSTM, data-parallel
over 8 NeuronCores.

Math (reference):
    h0 = enc @ W_enc + b_enc ; c0 = 0 ; x0 = fx @ W_emb + b_emb
    per step: gates = x @ W_k + h @ W_r + b_lstm  (i,f,cb,o)
              c' = sig(f)*c + sig(i)*tanh(cb) ; h' = sig(o)*tanh(c')
              y = h' @ W_red + b_red ; x' = y @ W_emb + b_emb
    out[:, t] = y_t

Host-side algebraic folds (exact, done in f64):
    x' feedback folds into the recurrence:  W_comb = W_r + W_red@W_emb@W_k,
        b_comb = b_lstm + (b_red*W_emb[0] + b_emb)@W_k
    step 0 folds the encoder projection:    gates_0 = enc@(W_enc@W_r) + fx@(W_emb@W_k) + b0
        b0 = b_lstm + b_emb@W_k + b_enc@W_r
    (h0 itself is never needed elementwise since c0 = 0.)

Device layout: state kept transposed (H on partitions, batch on free dim) so the
recurrent matmul needs no per-step transposes. The batch is processed in pairs
of 512-column chunks; each gate m-tile's preactivations for both chunks of a
pair live in one 2-bank PSUM tile so a single wide activation instruction
(with one per-partition bias) covers 1024 columns. y_t rows are accumulated
into persistent PSUM banks via shifted one-hot W_red columns. Output is
produced transposed [T, B_core] and untransposed on the host.

Dtypes: all gate matmuls (including the step-0 encoder projection) run in
float32r — the PE's native fast-fp32 mode, 1 cycle/row vs 4 for plain fp32,
~1e-4 relative error overall. The y projection uses f32r where the PSUM
destination starts at partition 0 and exact fp32 otherwise (f32r matmuls
reject nonzero dst base partitions).
"""

import numpy as np

P = 128
B, ENC, H, DE, T = 16384, 512, 256, 32, 64
NCORES = 8
BC = B // NCORES        # 2048 batch rows per core
CW = 512                # chunk width (PSUM bank = 512 fp32)
PW = 2 * CW             # chunk-pair width
KE = ENC // P           # 4 enc k-tiles
KH = H // P             # 2 hidden k-tiles
M4 = 4 * H // P         # 8 gate m-tiles

_NC_CACHE = {}


# repeats>1 re-runs the whole time loop (timing experiments only).
def _build_nc(bc=BC, t_steps=T, repeats=1):
    import concourse.bass as bass
    import concourse.tile as tile
    from concourse import bacc, mybir

    f32 = mybir.dt.float32
    f32r = mybir.dt.float32r
    bf16 = mybir.dt.bfloat16
    AF = mybir.ActivationFunctionType
    nch = bc // CW
    npair = bc // PW
    assert nch % 2 == 0
    # each chunk gets its own y bank at partition base 0 so every y matmul
    # can run in the fast f32r mode (f32r rejects nonzero dst base partition)
    n_ybanks = nch

    nc = bacc.Bacc("TRN2", target_bir_lowering=False, debug=False)
    encT_d = nc.declare_dram_parameter("enc_t", [ENC, bc], f32, isOutput=False)
    fxT_d = nc.declare_dram_parameter("fx_t", [1, bc], f32, isOutput=False)
    wer_d = nc.declare_dram_parameter("w_er", [ENC, 4 * H], f32, isOutput=False)
    wcomb_d = nc.declare_dram_parameter("w_comb", [H, 4 * H], f32, isOutput=False)
    wfk_d = nc.declare_dram_parameter("w_fk", [1, 4 * H], f32, isOutput=False)
    wredoh_d = nc.declare_dram_parameter(
        "w_red_oh", [P, KH, 2 * t_steps - 1], f32, isOutput=False)
    b0_d = nc.declare_dram_parameter("b0", [4 * H], f32, isOutput=False)
    bcomb_d = nc.declare_dram_parameter("b_comb", [4 * H], f32, isOutput=False)
    bred_d = nc.declare_dram_parameter("b_red_bc", [P, 1], f32, isOutput=False)
    ys_d = nc.declare_dram_parameter("ys_t", [t_steps, bc], f32, isOutput=True)

    with tile.TileContext(nc) as tc:
        with (
            tc.tile_pool(name="consts", bufs=1) as consts,
            tc.tile_pool(name="state", bufs=1) as state,
            tc.tile_pool(name="psum_g", bufs=2, space="PSUM") as psg,
            tc.tile_pool(name="psum_y", bufs=1, space="PSUM") as psy,
        ):
            gact_scope = tc.tile_pool(name="gact_p", bufs=2)
            gactp = gact_scope.__enter__()
            tmp_scope = tc.tile_pool(name="tmp_p", bufs=2)
            tmpp = tmp_scope.__enter__()
            # ---- constant loads ----
            # The staging pool is scoped and opened last so whatever reuses
            # its released zone is late-use, not the t0-critical pools.
            stage_scope = tc.tile_pool(name="stage", bufs=2)
            stgp = stage_scope.__enter__()

            def bounce(dst, src):
                # DMA fp32 bits into a staging tile, then DVE-copy into the
                # destination. The copy is a semaphore firewall (downstream
                # compute depends on one DVE semaphore, not many HW-DGE queue
                # semaphores; matmuls allow at most 2 sync waits and DMAs 1)
                # and performs the f32r rounding / bf16 downcast the consumers
                # need.
                stg = stgp.tile(list(dst.shape), f32, name="stg", tag="stg")
                nc.sync.dma_start(stg[:], src)
                nc.vector.tensor_copy(dst, stg[:])

            # step-0 inputs load first so the PE/ACT pipeline starts ASAP;
            # constants not needed until t>=1 (wcomb, bcomb) or the end
            # (wredoh, bred) load behind them.
            b0_sb = consts.tile([P, M4], f32, name="b0_sb")
            bounce(b0_sb[:], b0_d.rearrange("(mt p) -> p mt", p=P))
            wfk_sb = consts.tile([1, M4, P], f32r, name="wfk_sb")
            bounce(wfk_sb[:], wfk_d.rearrange("o (mt mp) -> o mt mp", mp=P))
            wer_sb = consts.tile([P, KE, M4, P], f32r, name="wer_sb")
            wer_v = wer_d.rearrange("(kt kp) (mt mp) -> kp kt mt mp", kp=P, mp=P)
            for k in range(KE):
                for mh in range(2):
                    bounce(wer_sb[:, k, mh * 4:(mh + 1) * 4],
                           wer_v[:, k, mh * 4:(mh + 1) * 4])
            encT_sb = consts.tile([P, KE, bc], f32r, name="encT_sb")
            encT_v = encT_d.rearrange("(kt p) n -> p kt n", p=P)
            fx_sb = consts.tile([1, bc], f32r, name="fx_sb")
            hw = bc // 2
            for half in range(2):
                sl = slice(half * hw, (half + 1) * hw)
                for k in range(KE):
                    bounce(encT_sb[:, k, sl], encT_v[:, k, sl])
                bounce(fx_sb[:, sl], fxT_d[:, sl])
            wcomb_sb = consts.tile([P, KH, M4, P], f32r, name="wcomb_sb")
            wcomb_v = wcomb_d.rearrange("(kt kp) (mt mp) -> kp kt mt mp", kp=P, mp=P)
            for k in range(KH):
                bounce(wcomb_sb[:, k], wcomb_v[:, k])
            bcomb_sb = consts.tile([P, M4], f32, name="bcomb_sb")
            bounce(bcomb_sb[:], bcomb_d.rearrange("(mt p) -> p mt", p=P))
            wredoh_sb = consts.tile([P, KH, 2 * t_steps - 1], f32r, name="wredoh_sb")
            bounce(wredoh_sb[:], wredoh_d[:])
            bred_sb = consts.tile([P, 1], f32, name="bred_sb")
            bounce(bred_sb[:], bred_d[:])

            stage_scope.__exit__(None, None, None)

            # ---- state (per chunk pair, transposed: H on partitions) ----
            # h feeds matmuls only, so it lives in the matmul input dtype.
            hs = [state.tile([P, KH, PW], f32r, name=f"hT_{p}") for p in range(npair)]
            cs = [state.tile([P, KH, PW], f32, name=f"cT_{p}") for p in range(npair)]
            for pi in range(npair):
                nc.vector.memset(cs[pi][:], 0.0)
            ybanks = [psy.tile([P, CW], f32, name=f"ybank_{i}") for i in range(n_ybanks)]

            # y_ty (row ty) accumulates into a persistent psum bank via the
            # shifted one-hot W_red weight (column ty of the sliding window).
            def emit_y(pi, ty, t_steps=t_steps):
                colw = slice(t_steps - 1 - ty, 2 * t_steps - 1 - ty)
                for j in range(2):
                    c = 2 * pi + j
                    yb = ybanks[c]
                    jcols = slice(j * CW, (j + 1) * CW)
                    for k in range(KH):
                        nc.tensor.matmul(
                            yb[0:t_steps, :], wredoh_sb[:, k, colw],
                            hs[pi][:, k, jcols],
                            start=(ty == 0 and k == 0),
                            stop=(ty == t_steps - 1 and k == KH - 1),
                            skip_group_check=True)

            # ---- time loop ----
            for _rep in range(repeats):
              for t in range(t_steps):
                for pi in range(npair):
                    if t > 0:
                        emit_y(pi, t - 1)
                    # per-pair gate preactivations: one 2-bank PSUM tile per
                    # m-tile holds both chunks, so each sigmoid/tanh covers
                    # 1024 columns with a single per-partition bias.
                    gact = gactp.tile([P, M4, PW], f32, name="gact", tag="gact")
                    for m in range(M4):
                        ps = psg.tile([P, 2, CW], f32, name="ps_g", tag="ps_g")
                        for j in range(2):
                            cols = slice((2 * pi + j) * CW, (2 * pi + j + 1) * CW)
                            jcols = slice(j * CW, (j + 1) * CW)
                            if t == 0:
                                for k in range(KE):
                                    nc.tensor.matmul(
                                        ps[:, j], wer_sb[:, k, m],
                                        encT_sb[:, k, cols],
                                        start=(k == 0), stop=False)
                                nc.tensor.matmul(
                                    ps[:, j], wfk_sb[:, m], fx_sb[:, cols],
                                    start=False, stop=True)
                            else:
                                nc.tensor.matmul(
                                    ps[:, j], wcomb_sb[:, 0, m],
                                    hs[pi][:, 0, jcols],
                                    start=True, stop=False)
                                nc.tensor.matmul(
                                    ps[:, j], wcomb_sb[:, 1, m],
                                    hs[pi][:, 1, jcols],
                                    start=False, stop=True)
                        func = AF.Tanh if m in (4, 5) else AF.Sigmoid
                        bias = (b0_sb if t == 0 else bcomb_sb)[:, m:m + 1]
                        nc.scalar.activation(
                            gact[:, m], ps.rearrange("p a b -> p (a b)"),
                            func, bias=bias)
                    # Elementwise cell update, per chunk within the pair so
                    # the next step's matmuls for chunk j start as soon as
                    # that chunk's h is ready (shorter cross-engine chain).
                    tmp = tmpp.tile([P, KH, PW], f32, name="tmp", tag="tmp")
                    tanhc = tmp
                    for j in range(2):
                        jc = slice(j * CW, (j + 1) * CW)
                        nc.vector.tensor_mul(
                            tmp[:, :, jc], gact[:, 0:KH, jc], gact[:, 4:4 + KH, jc])
                        nc.vector.tensor_mul(
                            cs[pi][:, :, jc], gact[:, 2:2 + KH, jc], cs[pi][:, :, jc])
                        nc.vector.tensor_add(
                            cs[pi][:, :, jc], cs[pi][:, :, jc], tmp[:, :, jc])
                        nc.scalar.activation(
                            tanhc[:, :, jc], cs[pi][:, :, jc], AF.Tanh)
                        nc.vector.tensor_mul(
                            hs[pi][:, :, jc], gact[:, 6:6 + KH, jc], tanhc[:, :, jc])
                    # (y matmuls for this h are emitted at the start of the
                    # next iteration — see emit_y — so the PE never queues an
                    # instruction that waits on this step's elementwise chain
                    # ahead of the next step's gates.)

            for pi in range(npair):
                emit_y(pi, t_steps - 1)

            # ---- drain y banks (add b_red) and store transposed output ----
            for c in range(nch):
                yb = ybanks[c]
                ys_sb = state.tile([P, CW], f32, name=f"ys_sb_{c}")
                nc.vector.tensor_scalar_add(
                    ys_sb[0:t_steps, :], yb[0:t_steps, :], bred_sb[0:t_steps, :])
                nc.sync.dma_start(
                    ys_d[:, c * CW:(c + 1) * CW], ys_sb[0:t_steps, :])

            tmp_scope.__exit__(None, None, None)
            gact_scope.__exit__(None, None, None)

    nc.finalize()
    return nc


def _get_nc():
    key = (BC, T)
    if key not in _NC_CACHE:
        _NC_CACHE[key] = _build_nc(*key)
    return _NC_CACHE[key]


def _prepare_in_maps(inputs):
    f64 = lambda a: np.asarray(a, np.float64)
    enc = np.asarray(inputs["encoded_input_series"], np.float32)
    fx = np.asarray(inputs["final_x_val"], np.float32)
    Wemb, bemb = f64(inputs["W_emb"]), f64(inputs["b_emb"])
    Wenc, benc = f64(inputs["W_enc"]), f64(inputs["b_enc"])
    Wk, Wr, blstm = f64(inputs["W_k"]), f64(inputs["W_r"]), f64(inputs["b_lstm"])
    Wred, bred = f64(inputs["W_red"]), f64(inputs["b_red"])
    t_steps = int(np.asarray(inputs["decode_length"]))
    assert t_steps == T and enc.shape == (B, ENC) and fx.shape == (B, 1)

    Wcomb = np.ascontiguousarray((Wr + Wred @ Wemb @ Wk), np.float32)
    bcomb = np.ascontiguousarray((blstm + (bred[0] * Wemb[0] + bemb) @ Wk), np.float32)
    Wer = np.ascontiguousarray((Wenc @ Wr), np.float32)
    Wfk = np.ascontiguousarray((Wemb @ Wk), np.float32)
    b0 = np.ascontiguousarray((blstm + bemb @ Wk + benc @ Wr), np.float32)
    Wred32 = np.asarray(Wred, np.float32)
    wredoh = np.zeros((P, KH, 2 * T - 1), np.float32)
    for k in range(KH):
        wredoh[:, k, T - 1] = Wred32[k * P:(k + 1) * P, 0]
    bred_bc = np.full((P, 1), bred[0], np.float32)

    in_maps = []
    for i in range(NCORES):
        sl = slice(i * BC, (i + 1) * BC)
        in_maps.append({
            "enc_t": np.ascontiguousarray(enc[sl].T),
            "fx_t": np.ascontiguousarray(fx[sl].reshape(1, BC)),
            "w_er": Wer,
            "w_comb": Wcomb,
            "w_fk": Wfk,
            "w_red_oh": wredoh,
            "b0": b0,
            "b_comb": bcomb,
            "b_red_bc": bred_bc,
        })
    return in_maps


def kernel(**inputs) -> np.ndarray:
    from concourse.bass_utils import run_bass_kernel_spmd

    in_maps = _prepare_in_maps(inputs)
    nc = _get_nc()
    res = run_bass_kernel_spmd(nc, in_maps, list(range(NCORES)))
    global LAST_RESULT
    LAST_RESULT = res
    ys_t = np.concatenate([res.results[i]["ys_t"] for i in range(NCORES)], axis=1)
    return np.ascontiguousarray(ys_t.T).astype(np.float32)


LAST_RESULT = None

